# revision 90
# baseline (speedup 1.0000x reference)
"""Trainium2 Bass kernel for nn_Decoder (2-layer bidirectional LSTM decoder,
autoregressive argmax feedback, T=512 steps, B=128, H=1024, V=64).

Strategy: 8-way tensor parallel over the 4H gate dimension. Each core holds a
512-wide slice of every gate projection (re-ordered [i,f,o,g] so activations
fuse), keeps the LSTM recurrence state resident, and exchanges the 128-row
h-slices it owns via two AllGathers per step. Matmuls run in fp16 (same
10-bit mantissa as TF32 for tanh-bounded h and ~N(0,1/32) weights, so
near-lossless vs the fp32r baseline) with the h-state (transposed) as the
stationary operand and the weight slice as the 512-wide moving operand;
PSUM accumulation stays fp32. fp16 halves both AllGather payloads and the
staging copies/DMAs around them. Per-core logits partials (with lin_b/8
folded in) also ride the second AllGather as packed fp16 pairs; the AG_B
staging tensors are declared F32 (NOT float32r -- the f32r path TF32-rounds
each 32-bit word, which silently destroys the low element of a packed
16-bit pair) with bitcast views for the fp16 regions. The summed logits
feed max_with_indices, whose argmax drives the next step's input on-device.

Scheduling notes:
- recurrent-gate (w_hh) partial matmuls are data-gated on the collective
  staging copies (a zero-valued scalar dependency) so the static Tile
  scheduler cannot place them ahead of the critical argmax->L0->stage or
  close->L1->stage chains; they then fill the AllGather windows.
- the h0 AllGather readback is split into 16 single-k-block DMAs so the 32
  dependent w_ih1 "close" matmuls release staggered in time, letting most of
  them be costed at the tensor engine's ramped (MID/PEAK) rates rather than
  the cold-start rate.
- the logits readback is split in two halves with a two-stage reduce, and the
  masked output store is gated behind the next stage's staging copy to keep
  it off the DVE critical window.
"""

import os
import sys

import numpy as np

sys.path.insert(0, "/opt/trn_rl_repo")

import concourse.bass as bass  # noqa: E402
import concourse.mybir as mybir  # noqa: E402
import concourse.tile as tile  # noqa: E402
from concourse import bacc  # noqa: E402
from concourse import bass_utils  # noqa: E402
from concourse.masks import make_identity  # noqa: E402

H = 1024
V = 64
B = 128
NCORES = 8
MASK_IDX = 4.0
KEEP_IDX = 3
T_STEPS = int(os.environ.get("DEC_T", "512"))
CHUNK = int(os.environ.get("DEC_CHUNK", "16"))
MM_DT = {"fp32r": mybir.dt.float32r, "fp32": mybir.dt.float32,
         "fp16": mybir.dt.float16}[os.environ.get("DEC_MMDT", "fp16")]
F32 = mybir.dt.float32
U32 = mybir.dt.uint32
LP16 = mybir.dt.float16  # packed logits-partial payload dtype
MMD = MM_DT
AF = mybir.ActivationFunctionType
ALU = mybir.AluOpType

# gate blocks packed per-core as [i, f, o, g] (torch order in rows is i,f,g,o)
GBASE = [0, H, 3 * H, 2 * H]

LOWPRI = 10_000_000
CH0_SIZES = [int(x) for x in os.environ.get("DEC_CH0", "1,1,1,1,4").split(",")]
NCH0 = len(CH0_SIZES)
NCH1 = 1   # h1T readback chunks per direction


def tf32_round(x):
    """Host-side rounding to the device matmul dtype (fp16 by default:
    same 10-bit mantissa as TF32, and native byte semantics on the wire)."""
    if MM_DT == mybir.dt.float16:
        return np.asarray(x, np.float16)
    if MM_DT == F32:
        return np.asarray(x, np.float32)
    xi = np.asarray(x, np.float32).view(np.uint32)
    xi = (xi + np.uint32(1 << 12)) & np.uint32(0xFFFFE000)
    return xi.view(np.float32)


def build(T=T_STEPS):
    nc = bacc.Bacc("TRN2", num_devices=NCORES)
    RG = [list(range(NCORES))]
    K0 = 8 // NCH0  # k-blocks per h0 chunk
    K1 = 8 // NCH1

    din = dict(kind="ExternalInput")
    w0T = nc.dram_tensor("w0T", [2, 128, 8, 512], MMD, **din)
    w0aug = nc.dram_tensor("w0aug", [2, 2, 512], MMD, **din)
    w1iT = nc.dram_tensor("w1iT", [2, 128, 16, 512], MMD, **din)
    w1hT = nc.dram_tensor("w1hT", [2, 128, 8, 512], MMD, **din)
    b1row = nc.dram_tensor("b1row", [2, 1, 512], MMD, **din)
    linTc = nc.dram_tensor("linTc", [128, 2, 64], MMD, **din)
    linbrow = nc.dram_tensor("linbrow", [1, 64], MMD, **din)
    notkeep = nc.dram_tensor("notkeep", [128, 64], F32, **din)
    hT0 = nc.dram_tensor("hT0", [4, 128, 8, 128], MMD, **din)
    c0s = nc.dram_tensor("c0s", [4, 128, 128], F32, **din)
    onesrow = nc.dram_tensor("onesrow", [1, 128], MMD, **din)
    x0row = nc.dram_tensor("x0row", [1, 128], MMD, **din)
    flag0 = nc.dram_tensor("flag0", [128, 1], F32, **din)
    hT_f = nc.dram_tensor("hT_f", [4, 128, 8, 128], MMD, kind="ExternalOutput")
    c_f = nc.dram_tensor("c_f", [4, 128, 128], F32, kind="ExternalOutput")
    flag_f = nc.dram_tensor("flag_f", [128, 1], F32, kind="ExternalOutput")
    idx_f = nc.dram_tensor("idx_f", [128, 1], F32, kind="ExternalOutput")
    y = nc.dram_tensor("y", [B, T, V], F32, kind="ExternalOutput")

    with tile.TileContext(nc) as tc:
        import contextlib

        ctx = contextlib.ExitStack()
        with ctx:
            wp = ctx.enter_context(tc.tile_pool(name="weights", bufs=1))
            hp = ctx.enter_context(tc.tile_pool(name="hstate", bufs=2))
            cp = ctx.enter_context(tc.tile_pool(name="cstate", bufs=2))
            gp = ctx.enter_context(tc.tile_pool(name="gact", bufs=1))
            ewp = ctx.enter_context(tc.tile_pool(name="ew", bufs=2))
            sp = ctx.enter_context(tc.tile_pool(name="send", bufs=2))
            ap_ = ctx.enter_context(tc.tile_pool(name="amax", bufs=2))
            yp = ctx.enter_context(tc.tile_pool(name="ybuf", bufs=1))
            pg = ctx.enter_context(tc.tile_pool(name="pgates", bufs=1, space="PSUM"))
            pt = ctx.enter_context(tc.tile_pool(name="ptrans", bufs=1, space="PSUM"))
            px = ctx.enter_context(tc.tile_pool(name="pmisc", bufs=1, space="PSUM"))
            dp = ctx.enter_context(tc.tile_pool(name="dram", bufs=2, space="DRAM"))

            # ---- load weights + constants (once) ----
            w0_sb, w0a_sb, w1i_sb, w1h_sb, b1_sb = [], [], [], [], []
            for d in range(2):
                t_ = wp.tile([128, 8, 512], MMD, tag=f"w0_{d}")
                nc.sync.dma_start(out=t_[:], in_=w0T[d])
                w0_sb.append(t_)
                tb = wp.tile([1, 512], MMD, tag=f"w0b_{d}")
                nc.sync.dma_start(out=tb[:], in_=w0aug[d, 1:2])
                tx = wp.tile([1, 512], MMD, tag=f"w0x_{d}")
                nc.sync.dma_start(out=tx[:], in_=w0aug[d, 0:1])
                w0a_sb.append((tx, tb))
                t_ = wp.tile([128, 16, 512], MMD, tag=f"w1i_{d}")
                nc.sync.dma_start(out=t_[:], in_=w1iT[d])
                w1i_sb.append(t_)
                t_ = wp.tile([128, 8, 512], MMD, tag=f"w1h_{d}")
                nc.sync.dma_start(out=t_[:], in_=w1hT[d])
                w1h_sb.append(t_)
                t_ = wp.tile([1, 512], MMD, tag=f"b1_{d}")
                nc.sync.dma_start(out=t_[:], in_=b1row[d])
                b1_sb.append(t_)
            lin_sb = wp.tile([128, 2, 64], MMD, tag="lin")
            nc.sync.dma_start(out=lin_sb[:], in_=linTc[:])
            linb_sb = wp.tile([1, 64], MMD, tag="linb")
            nc.sync.dma_start(out=linb_sb[:], in_=linbrow[:])
            nk_sb = wp.tile([128, 64], F32, tag="nk")
            nc.sync.dma_start(out=nk_sb[:], in_=notkeep[:])
            ident = wp.tile([128, 128], F32, tag="ident")
            make_identity(nc, ident[:])
            ones = wp.tile([1, 128], MMD, tag="ones")
            nc.sync.dma_start(out=ones[:], in_=onesrow[:])

            # ---- initial state (h state chunked to match readback tiling) --
            h_prev = []
            for cell in range(2):
                chks = []
                koff = 0
                for ch, ksz in enumerate(CH0_SIZES):
                    t_ = hp.tile([128, ksz, 128], MMD, tag=f"h{cell}_{ch}")
                    nc.sync.dma_start(
                        out=t_[:], in_=hT0[cell][:, koff:koff + ksz, :])
                    chks.append((t_, koff, ksz))
                    koff += ksz
                h_prev.append(chks)
            for cell in range(2, 4):
                chks = []
                for ch in range(NCH1):
                    t_ = hp.tile([128, K1, 128], MMD, tag=f"h{cell}_{ch}")
                    nc.sync.dma_start(
                        out=t_[:], in_=hT0[cell][:, ch * K1:(ch + 1) * K1, :])
                    chks.append(t_)
                h_prev.append(chks)
            c_prev = []
            for cell in range(4):
                t_ = cp.tile([128, 128], F32, tag=f"c{cell}")
                nc.sync.dma_start(out=t_[:], in_=c0s[cell])
                c_prev.append(t_)
            flag_prev = ap_.tile([128, 1], F32, tag="flag")
            nc.sync.dma_start(out=flag_prev[:], in_=flag0[:])
            x_row = ap_.tile([1, 128], MMD, tag="xrow")
            nc.sync.dma_start(out=x_row[:], in_=x0row[:])

            def h0_kblk(hlist, d, k):
                for t_, koff, ksz in hlist[d]:
                    if koff <= k < koff + ksz:
                        return t_[:, k - koff, :]
                raise AssertionError

            def h1_kblk(hlist, d, k):
                return hlist[2 + d][k // K1][:, k % K1, :]

            # priming: g0 / g1 partials for step 0
            g0 = []
            for d in range(2):
                g = pg.tile([128, 512], F32, tag=f"g0{d}", name=f"g0p_{d}")
                nc.tensor.matmul(g[:], (ones[:]), (w0a_sb[d][1][:]),
                                 start=True, stop=False)
                for k in range(8):
                    nc.tensor.matmul(g[:], h0_kblk(h_prev, d, k),
                                     (w0_sb[d][:, k, :]),
                                     start=False, stop=False)
                g0.append(g)
            g1 = []
            for d in range(2):
                g = pg.tile([128, 512], F32, tag=f"g1{d}", name=f"g1p_{d}")
                nc.tensor.matmul(g[:], (ones[:]), (b1_sb[d][:]),
                                 start=True, stop=False)
                for k in range(8):
                    nc.tensor.matmul(g[:], h1_kblk(h_prev, d, k),
                                     (w1h_sb[d][:, k, :]),
                                     start=False, stop=False)
                g1.append(g)

            ybuf = None
            agB_out_prev = None
            idx = None
            fnew = None

            def consume_agB(t):
                """Read back h1T, compute logits + argmax + flag + y store for
                step t-1 from the gathered h1. Returns (h1chunks, idx, fnew)."""
                nonlocal ybuf
                # logits partials ride the payload as packed bf16 pairs in 32
                # f32r slots; the DMA moves plain f32r words and the bf16
                # view is only taken on the SBUF tile for the reduce
                LG = ap_.tile([128, 8, 32], F32, tag="LG")
                for hf, (o0, o1) in enumerate(((0, 6), (6, 8))):
                    nc.sync.dma_start(
                        out=LG[:, o0:o1, :],
                        in_=agB_out_prev[o0 * 128:o1 * 128,
                                         128:160].rearrange(
                            "(c p) v -> p c v", p=128),
                    )
                Lh = ap_.tile([128, 64], F32, tag="Lh")
                nc.vector.tensor_reduce(Lh[:],
                                        LG[:, 0:6, :].bitcast(LP16).rearrange(
                                            "p c v -> p v c"),
                                        axis=mybir.AxisListType.X,
                                        op=ALU.add)
                L2 = ap_.tile([128, 64], F32, tag="L2")
                nc.vector.tensor_reduce(L2[:],
                                        LG[:, 6:8, :].bitcast(LP16).rearrange(
                                            "p c v -> p v c"),
                                        axis=mybir.AxisListType.X,
                                        op=ALU.add)
                L = ap_.tile([128, 64], F32, tag="L")
                nc.vector.tensor_add(L[:], Lh[:], L2[:])
                h1ch = [[], []]
                for d in range(2):
                    for ch in range(NCH1):
                        t_ = hp.tile([128, K1, 128], MMD, tag=f"h{2 + d}_{ch}")
                        nc.sync.dma_start(
                            out=t_[:],
                            in_=agB_out_prev[ch * K1 * 128:(ch + 1) * K1 * 128,
                                             d * 64:(d + 1) * 64].bitcast(
                                MMD).rearrange("(k p) b -> p k b", p=128),
                        )
                        h1ch[d].append(t_)
                mx8 = ap_.tile([128, 8], F32, tag="mx8")
                mi8 = ap_.tile([128, 8], U32, tag="mi8")
                nc.vector.max_with_indices(mx8[:], mi8[:], L[:])
                idx_ = ap_.tile([128, 1], F32, tag="idx")
                nc.vector.tensor_copy(idx_[:], mi8[:, 0:1])
                # flag + masked output store (off critical path)
                with tc.high_priority(offset=-LOWPRI):
                    flagb = ap_.tile([128, 1], F32, tag="flagb")
                    nc.vector.tensor_scalar(flagb[:], idx_[:], 1.0, None,
                                            op0=ALU.is_equal)
                    fnew_ = ap_.tile([128, 1], F32, tag="flag")
                    nc.vector.tensor_max(fnew_[:], flag_prev[:], flagb[:])
                return h1ch, idx_, fnew_, L

            def store_y(t, L, fnew_, gate):
                nonlocal ybuf
                with tc.high_priority(offset=-LOWPRI):
                    zz = ap_.tile([128, 1], F32, tag="zz")
                    gv = (gate[:, 0:1] if mybir.dt.size(gate.dtype) == 4
                          else gate[:, 0:2].bitcast(F32))
                    nc.vector.tensor_scalar(zz[:], gv,
                                            0.0, None, op0=ALU.mult)
                    tk = ap_.tile([128, 64], F32, tag="tk")
                    nc.vector.tensor_scalar(tk[:], nk_sb[:], zz[:], None,
                                            op0=ALU.add)
                    nc.vector.tensor_mul(tk[:], L[:], tk[:])
                    tk2 = ap_.tile([128, 64], F32, tag="tk2")
                    nc.vector.tensor_scalar(tk2[:], tk[:], fnew_[:], None,
                                            op0=ALU.mult)
                    s = t - 1
                    if s % CHUNK == 0:
                        ybuf = yp.tile([128, CHUNK, 64], F32, tag="ybuf")
                    nc.vector.tensor_sub(ybuf[:, s % CHUNK, :], L[:], tk2[:])
                    if s % CHUNK == CHUNK - 1:
                        nc.sync.dma_start(out=y[:, s - CHUNK + 1:s + 1, :],
                                          in_=ybuf[:])

            for t in range(T):
                # ================= c2 segment =================
                if agB_out_prev is not None:
                    h1ch, idx, fnew, Lt = consume_agB(t)
                    flag_prev = fnew
                    h_prev[2], h_prev[3] = h1ch[0], h1ch[1]
                    # x row for this step
                    x_ps = px.tile([1, 128], F32, tag="xps")
                    nc.tensor.transpose(x_ps[:], idx[:], ident[:])
                    x_row = ap_.tile([1, 128], MMD, tag="xrow")
                    nc.vector.tensor_copy(x_row[:], x_ps[:])

                # close g0 with x contribution
                for d in range(2):
                    nc.tensor.matmul(g0[d][:], (x_row[:]),
                                     (w0a_sb[d][0][:]),
                                     start=False, stop=True)
                # L0 elementwise
                ptA = pt.tile([128, 256], F32, tag="ptA")
                agA_in = dp.tile([128, 256], MMD, tag="agAi")
                c_new = [None] * 4
                for d in range(2):
                    a = gp.tile([128, 512], F32, tag=f"a{d}", name=f"aL0_{d}")
                    nc.scalar.activation(a[:, 0:384], g0[d][:, 0:384], AF.Sigmoid)
                    nc.scalar.activation(a[:, 384:512], g0[d][:, 384:512], AF.Tanh)
                    t1 = ewp.tile([128, 128], F32, tag="t1")
                    nc.vector.tensor_mul(t1[:], a[:, 128:256], c_prev[d][:])
                    t2 = ewp.tile([128, 128], F32, tag="t2")
                    nc.vector.tensor_mul(t2[:], a[:, 0:128], a[:, 384:512])
                    cn = cp.tile([128, 128], F32, tag=f"c{d}")
                    nc.vector.tensor_add(cn[:], t1[:], t2[:])
                    tc2 = ewp.tile([128, 128], F32, tag="tc2")
                    nc.scalar.activation(tc2[:], cn[:], AF.Tanh)
                    h2 = gp.tile([128, 128], F32, tag=f"h2_{d}")
                    nc.vector.tensor_mul(h2[:], a[:, 256:384], tc2[:])
                    c_new[d] = cn
                    nc.tensor.transpose(ptA[:, d * 128:(d + 1) * 128], h2[:],
                                        ident[:])
                sendA = sp.tile([128, 256], MMD, tag="sendA")
                nc.vector.tensor_copy(sendA[:, 0:128], ptA[:, 0:128])
                nc.vector.tensor_copy(sendA[:, 128:256], ptA[:, 128:256])
                nc.sync.dma_start(out=agA_in[:], in_=sendA[:])
                if agB_out_prev is not None:
                    store_y(t, Lt, fnew, sendA)

                # g1 hh partials for THIS step, gated on sendA so the static
                # scheduler cannot place them ahead of the critical c2 chain;
                # they then fill the AG_A window.
                with tc.high_priority(offset=-LOWPRI):
                    g1n = []
                    if t > 0:
                        zg = ap_.tile([1, 1], F32, tag="zgA")
                        nc.vector.tensor_scalar(zg[:], sendA[0:1, 0:2].bitcast(F32),
                                                0.0, None, op0=ALU.mult)
                        onesgA = ap_.tile([1, 128], MMD, tag="onesgA")
                        nc.vector.tensor_scalar(onesgA[:], ones[:], zg[:],
                                                None, op0=ALU.add)
                    for d in range(2):
                        g = pg.tile([128, 512], F32, tag=f"g1{d}",
                                    name=f"g1n_{d}") if t > 0 else g1[d]
                        if t > 0:
                            nc.tensor.matmul(g[:], (onesgA[:]), (b1_sb[d][:]),
                                             start=True, stop=False)
                            for k in range(8):
                                nc.tensor.matmul(g[:], h1_kblk(h_prev, d, k),
                                                 (w1h_sb[d][:, k, :]),
                                                 start=False, stop=False)
                        g1n.append(g)
                    g1 = g1n

                # ============== AG_A ==============
                agA_out = dp.tile([1024, 256], MMD, tag="agAo",
                                  addr_space="Shared")
                nc.gpsimd.collective_compute(
                    "AllGather", ALU.bypass, replica_groups=RG,
                    ins=[agA_in.opt()], outs=[agA_out.opt()],
                )
                # ============== c1 segment ==============
                # chunked readback staggers the closes' release times
                h0T_new = [[], []]
                for sd in range(2):
                    koff = 0
                    for ch, ksz in enumerate(CH0_SIZES):
                        t_ = hp.tile([128, ksz, 128], MMD, tag=f"h{sd}_{ch}")
                        nc.sync.dma_start(
                            out=t_[:],
                            in_=agA_out[koff * 128:(koff + ksz) * 128,
                                        sd * 128:(sd + 1) * 128].rearrange(
                                "(k p) b -> p k b", p=128),
                        )
                        h0T_new[sd].append((t_, koff, ksz))
                        koff += ksz
                # close L1 gates: w_ih1 over gathered h0, chunk-major;
                # the final chunk runs d-major so dir0's group closes early
                # and sigma_0 overlaps dir1's remaining closes
                for sd in range(2):
                    for ch in range(NCH0):
                        t_, koff, ksz = h0T_new[sd][ch]
                        final = sd == 1 and ch == NCH0 - 1
                        if final:
                            for d in range(2):
                                for k in range(ksz):
                                    kb = sd * 8 + koff + k
                                    last = k == ksz - 1
                                    nc.tensor.matmul(
                                        g1[d][:], (t_[:, k, :]),
                                        (w1i_sb[d][:, kb, :]),
                                        start=False, stop=last,
                                    )
                        else:
                            for k in range(ksz):
                                for d in range(2):
                                    nc.tensor.matmul(
                                        g1[d][:], (t_[:, k, :]),
                                        (w1i_sb[d][:, sd * 8 + koff + k, :]),
                                        start=False, stop=False,
                                    )
                # L1 elementwise + transpose
                ptB = pt.tile([128, 256], F32, tag="ptB")
                sendBh = sp.tile([128, 256], MMD, tag="sendBh")
                agB_in = dp.tile([128, 160], F32, tag="agBi")
                for d in range(2):
                    a = gp.tile([128, 512], F32, tag=f"a{d}", name=f"aL1_{d}")
                    nc.scalar.activation(a[:, 0:384], g1[d][:, 0:384], AF.Sigmoid)
                    nc.scalar.activation(a[:, 384:512], g1[d][:, 384:512], AF.Tanh)
                    t1 = ewp.tile([128, 128], F32, tag="t1")
                    nc.vector.tensor_mul(t1[:], a[:, 128:256], c_prev[2 + d][:])
                    t2 = ewp.tile([128, 128], F32, tag="t2")
                    nc.vector.tensor_mul(t2[:], a[:, 0:128], a[:, 384:512])
                    cn = cp.tile([128, 128], F32, tag=f"c{2 + d}")
                    nc.vector.tensor_add(cn[:], t1[:], t2[:])
                    tc2 = ewp.tile([128, 128], F32, tag="tc2")
                    nc.scalar.activation(tc2[:], cn[:], AF.Tanh)
                    h2 = gp.tile([128, 128], F32, tag=f"h2_{2 + d}")
                    nc.vector.tensor_mul(h2[:], a[:, 256:384], tc2[:])
                    c_new[2 + d] = cn
                    nc.tensor.transpose(ptB[:, d * 128:(d + 1) * 128], h2[:],
                                        ident[:])
                nc.vector.tensor_copy(sendBh[:, 0:128], ptB[:, 0:128])
                nc.vector.tensor_copy(sendBh[:, 128:256], ptB[:, 128:256])
                nc.sync.dma_start(out=agB_in[:, 0:128].bitcast(MMD),
                                  in_=sendBh[:])
                lp = px.tile([128, 64], F32, tag="lp")
                nc.tensor.matmul(lp[:], (ones[:]), (linb_sb[:]),
                                 start=True, stop=False)
                for d in range(2):
                    nc.tensor.matmul(lp[:], (sendBh[:, d * 128:(d + 1) * 128]),
                                     (lin_sb[:, d, :]),
                                     start=False, stop=(d == 1))
                sendBl = sp.tile([128, 64], LP16, tag="sendBl")
                nc.vector.tensor_copy(sendBl[:], lp[:])
                nc.sync.dma_start(out=agB_in[:, 128:160],
                                  in_=sendBl[:].bitcast(F32))

                # g0 partials for NEXT step, gated on sendBh so they fill the
                # AG_B window instead of delaying the L1 staging chain.
                with tc.high_priority(offset=-LOWPRI):
                    zg = ap_.tile([1, 1], F32, tag="zgB")
                    nc.vector.tensor_scalar(zg[:], sendBh[0:1, 0:2].bitcast(F32),
                                            0.0, None, op0=ALU.mult)
                    onesgB = ap_.tile([1, 128], MMD, tag="onesgB")
                    nc.vector.tensor_scalar(onesgB[:], ones[:], zg[:],
                                            None, op0=ALU.add)
                    g0n = []
                    for d in range(2):
                        g = pg.tile([128, 512], F32, tag=f"g0{d}",
                                    name=f"g0n_{d}")
                        nc.tensor.matmul(g[:], (onesgB[:]), (w0a_sb[d][1][:]),
                                         start=True, stop=False)
                        for ch in range(NCH0):
                            t_, koff, ksz = h0T_new[d][ch]
                            for k in range(ksz):
                                nc.tensor.matmul(g[:], (t_[:, k, :]),
                                                 (w0_sb[d][:, koff + k, :]),
                                                 start=False, stop=False)
                        g0n.append(g)
                    g0 = g0n

                # ============== AG_B ==============
                agB_out = dp.tile([1024, 160], F32, tag="agBo",
                                  addr_space="Shared")
                nc.gpsimd.collective_compute(
                    "AllGather", ALU.bypass, replica_groups=RG,
                    ins=[agB_in.opt()], outs=[agB_out.opt()],
                )
                agB_out_prev = agB_out

                # carry
                h_prev[0], h_prev[1] = h0T_new[0], h0T_new[1]
                c_prev = c_new

            # ---- final: consume last AG_B for idx/flag/output/h1 state ----
            h1ch, idx, fnew, Lt = consume_agB(T)
            store_y(T, Lt, fnew, idx)
            h_prev[2], h_prev[3] = h1ch[0], h1ch[1]
            if (T - 1) % CHUNK != CHUNK - 1:
                nfin = (T - 1) % CHUNK + 1
                nc.sync.dma_start(out=y[:, T - nfin:T, :], in_=ybuf[:, 0:nfin, :])
            for cell in range(4):
                if cell < 2:
                    for t_, koff, ksz in h_prev[cell]:
                        nc.sync.dma_start(
                            out=hT_f[cell][:, koff:koff + ksz, :], in_=t_[:])
                else:
                    kc = 8 // NCH1
                    for ch in range(NCH1):
                        nc.sync.dma_start(
                            out=hT_f[cell][:, ch * kc:(ch + 1) * kc, :],
                            in_=h_prev[cell][ch][:])
                nc.sync.dma_start(out=c_f[cell], in_=c_prev[cell][:])
            nc.sync.dma_start(out=flag_f[:], in_=fnew[:])
            nc.sync.dma_start(out=idx_f[:], in_=idx[:])
    nc.finalize()
    return nc


def prep_inputs(h0, c0, w_ih0, w_hh0, b0, w_ih1, w_hh1, b1, lin_w, lin_b):
    """Host-side packing: per-core sliced/transposed weight + state arrays."""
    h0 = np.asarray(h0, np.float32).reshape(2, 2, B, H)
    c0 = np.asarray(c0, np.float32).reshape(2, 2, B, H)
    w_ih0 = np.asarray(w_ih0, np.float32)
    w_hh0 = np.asarray(w_hh0, np.float32)
    b0 = np.asarray(b0, np.float32)
    w_ih1 = np.asarray(w_ih1, np.float32)
    w_hh1 = np.asarray(w_hh1, np.float32)
    b1 = np.asarray(b1, np.float32)
    lin_w = np.asarray(lin_w, np.float32)
    lin_b = np.asarray(lin_b, np.float32)

    nk = np.ones((128, V), np.float32)
    nk[:, KEEP_IDX] = 0.0

    hT0 = np.zeros((4, 128, 8, B), np.float32)
    for l in range(2):
        for d in range(2):
            cell = l * 2 + d
            hT0[cell] = h0[l, d].T.reshape(8, 128, B).transpose(1, 0, 2)

    in_maps = []
    for c in range(NCORES):
        rows = np.concatenate([np.arange(gb + c * 128, gb + c * 128 + 128)
                               for gb in GBASE])

        def packT(w, kt):
            # w: (4H, K*128) -> select rows -> [p, k, n]
            sel = w[rows, :]  # (512, kt*128)
            return np.ascontiguousarray(
                sel.reshape(512, kt, 128).transpose(2, 1, 0))

        w0T = np.stack([packT(w_hh0[d], 8) for d in range(2)])
        w1iT = np.stack([packT(w_ih1[d], 16) for d in range(2)])
        w1hT = np.stack([packT(w_hh1[d], 8) for d in range(2)])
        w0aug = np.stack([np.stack([w_ih0[d][rows, 0], b0[d][rows]])
                          for d in range(2)])
        b1row = np.stack([b1[d][rows][None, :] for d in range(2)])
        linTc = np.stack(
            [lin_w[:, c * 128:(c + 1) * 128].T,
             lin_w[:, H + c * 128:H + (c + 1) * 128].T], axis=1)
        c0slice = np.zeros((4, 128, 128), np.float32)
        for l in range(2):
            for d in range(2):
                c0slice[l * 2 + d] = c0[l, d][:, c * 128:(c + 1) * 128]
        in_maps.append({
            "w0T": tf32_round(np.ascontiguousarray(w0T)),
            "w0aug": tf32_round(np.ascontiguousarray(w0aug)),
            "w1iT": tf32_round(np.ascontiguousarray(w1iT)),
            "w1hT": tf32_round(np.ascontiguousarray(w1hT)),
            "b1row": tf32_round(np.ascontiguousarray(b1row)),
            "linTc": tf32_round(np.ascontiguousarray(linTc)),
            "linbrow": tf32_round(lin_b[None, :] / NCORES),
            "notkeep": nk,
            "hT0": tf32_round(hT0),
            "c0s": np.ascontiguousarray(c0slice),
            "onesrow": tf32_round(np.ones((1, 128), np.float32)),
            "x0row": tf32_round(np.full((1, 128), MASK_IDX, np.float32)),
            "flag0": np.zeros((128, 1), np.float32),
        })
    return in_maps


_NC_CACHE = {}


def _get_nc(T):
    if T not in _NC_CACHE:
        _NC_CACHE[T] = build(T)
    return _NC_CACHE[T]


T_LAUNCH = int(os.environ.get("DEC_TLAUNCH", "256"))


def kernel(h0, c0, w_ih0, w_hh0, b0, w_ih1, w_hh1, b1, lin_w, lin_b,
           decoder_output_length, batch_size, _want_results=False):
    T = int(decoder_output_length)
    assert int(batch_size) == B
    in_maps = prep_inputs(h0, c0, w_ih0, w_hh0, b0, w_ih1, w_hh1, b1,
                          lin_w, lin_b)
    chunks = []
    t_done = 0
    res = None
    while t_done < T:
        t_this = min(T_LAUNCH, T - t_done)
        nc = _get_nc(t_this)
        res = bass_utils.run_bass_kernel_spmd(nc, in_maps,
                                              core_ids=list(range(NCORES)))
        chunks.append(res.results[0]["y"])
        t_done += t_this
        if t_done < T:
            idxs = res.results[0]["idx_f"]  # (128,1) float indices
            xrow = tf32_round(np.ascontiguousarray(idxs.reshape(1, 128)))
            for c in range(NCORES):
                rc = res.results[c]
                in_maps[c] = dict(in_maps[c])
                in_maps[c]["hT0"] = rc["hT_f"]
                in_maps[c]["c0s"] = rc["c_f"]
                in_maps[c]["flag0"] = rc["flag_f"]
                in_maps[c]["x0row"] = xrow
    out = np.concatenate(chunks, axis=1) if len(chunks) > 1 else chunks[0]
    if _want_results:
        return out, res
    return out

